# revision 32
# baseline (speedup 1.0000x reference)
"""Fused self-attention (FCSelfAttention) Trainium2 Bass kernel.

Problem: X:[4,2048,512] fp32, W_qkv:[512,1536], W_out:[512,512], b_out:[512]
  qkv = X @ W_qkv ; q,k,v -> heads (B,H=8,N=2048,DH=64)
  scores[n,m] = k_n . q_m * DH**-0.5 ; softmax over m (query axis)
  out[n] = sum_e att[n,e] v[e] ; merge heads ; @ W_out + b_out

Sharding (8 cores): batch x head-group. Core c handles batch b=c//2 and
heads 4g..4g+3 where g=c%2 (data parallel over B=4, tensor parallel over
H=8 in halves). Each core computes a partial output projection for its
batch; the host sums the two partials per batch and adds b_out.

Device algorithm (per core), flash-style with scores kept transposed so
the softmax axis lands on the TensorE contraction axis:
  S^T[m,n] = sum_d QT[d,m] KT[d,n]        (m = softmax axis, on partitions)
  P^T = exp(S^T * SCALE)                   (no max subtraction; |S| < 9)
  PV:  lhsT = V_aug[e, 0:65] (col 64 = ones) -> psum[0:64]=out^T, psum[64]=Z

v2 changes over the first working kernel:
  - software pipelining: the score matmuls for block g+1 are emitted
    BEFORE block g's PV matmuls, so the (in-order) PE always has the
    next exp's input ready early and ACT never waits on PE backlog.
  - two of every 16 m-blocks compute exp on the DVE instead of ACT via
    the integer exp trick (bits of bf16 = s*16*log2e + 16250, computed
    by one tensor_scalar with int16 output); this offloads ~12.5% of
    the exp stream off the pacing Scalar engine at ~0.4% output error.
  - staged startup: only the tensors needed by quarter 0 gate the first
    exp; everything else (xt tail, remaining V/projection pieces) is
    drained through the per-block pending queue.
  - no DMAs ride the Scalar queue (they'd serialize with ACTIVATEs).
"""

import sys

import numpy as np

_B, _N, _DIM = 4, 2048, 512
_H, _DH = 8, 64
_SCALE = _DH ** -0.5
_NCORES = 8
_HPC = 4              # heads per core
_HL = _HPC * _DH      # 256 local inner dim
_TC = _N // 128       # 16 token chunks
_KC = _DIM // 128     # 4 contraction chunks for projections

# DVE integer-exp constants: bf16 bits of exp(s*SCALE) ~= s*A + B
_EXPA = 128.0 * np.log2(np.e) * _SCALE      # 16*log2(e)
_EXPB = 127.0 * 128.0 - 6.0                  # bias, c=-6 tuned offline
_DVE_ECS = (5, 11)                           # m-blocks per quarter on DVE

_cache = {}


def _emit(tc, xt, wq, wk, wv, wo, out, mybir):
    nc = tc.nc
    dt = mybir.dt
    f32, bf16, i16 = dt.float32, dt.bfloat16, dt.int16
    Exp = mybir.ActivationFunctionType.Exp
    Copy = mybir.ActivationFunctionType.Copy
    Alu = mybir.AluOpType

    from contextlib import ExitStack

    with ExitStack() as ctx:
        weights = ctx.enter_context(tc.tile_pool(name="weights", bufs=1))
        xtp = ctx.enter_context(tc.tile_pool(name="xtp", bufs=1))
        qkp = ctx.enter_context(tc.tile_pool(name="qkp", bufs=1))
        vap = ctx.enter_context(tc.tile_pool(name="vap", bufs=1))
        atp = ctx.enter_context(tc.tile_pool(name="atp", bufs=1))
        ptp = ctx.enter_context(tc.tile_pool(name="ptp", bufs=5))
        zp = ctx.enter_context(tc.tile_pool(name="zp", bufs=2))
        zdp = ctx.enter_context(tc.tile_pool(name="zdp", bufs=2, space="DRAM"))
        outp = ctx.enter_context(tc.tile_pool(name="outp", bufs=1))
        psA = ctx.enter_context(tc.tile_pool(name="psA", bufs=2, space="PSUM"))
        psOp = ctx.enter_context(tc.tile_pool(name="psO", bufs=2, space="PSUM"))
        psB = ctx.enter_context(tc.tile_pool(name="psB", bufs=2, space="PSUM"))

        # ---- input DMAs, staged ------------------------------------------
        # Contiguous per-chunk DMAs (rearranged single DMAs generate 512B
        # scattered descriptors, ~5x slower), spread across the sync /
        # gpsimd / vector queues.  Stage A (xt cols 0:512 + all weights)
        # gates the first exp; stages B/C land during quarter 0.
        ones11 = weights.tile([1, 1], f32, tag="ones11", name="ones11")
        nc.vector.memset(ones11, 1.0)
        dummy = xtp.tile([128, 512], bf16, tag="dummy", name="dummy")
        nc.vector.memset(dummy, 0.0)
        xt_sb = []
        for kc in range(_KC):
            t = xtp.tile([128, _N], bf16, tag=f"xt{kc}", name=f"xt{kc}")
            xt_sb.append(t)

        # gating tensors first on each queue: gpsimd: wq then xtA(2);
        # sync: xtA(2) then wk then xt-B; scalar(idle at start): wv, wo.
        wq_sb, wk_sb, wv_sb = [], [], []
        for kc in range(_KC):
            t = weights.tile([128, _HL], bf16, tag=f"wq{kc}", name=f"wq{kc}")
            nc.gpsimd.dma_start(t, wq[kc * 128:(kc + 1) * 128, :])
            wq_sb.append(t)
        for kc in range(_KC):
            (nc.sync if kc < 2 else nc.gpsimd).dma_start(
                xt_sb[kc][:, 0:512], xt[kc * 128:(kc + 1) * 128, 0:512])
        for kc in range(_KC):
            t = weights.tile([128, _HL], bf16, tag=f"wk{kc}", name=f"wk{kc}")
            nc.sync.dma_start(t, wk[kc * 128:(kc + 1) * 128, :])
            wk_sb.append(t)
        for kc in range(_KC):
            t = weights.tile([128, _HL], bf16, tag=f"wv{kc}", name=f"wv{kc}")
            nc.scalar.dma_start(t, wv[kc * 128:(kc + 1) * 128, :])
            wv_sb.append(t)
        wo_sb = []
        for pair in range(2):
            t = weights.tile([128, _DIM], bf16, tag=f"wo{pair}",
                             name=f"wo{pair}")
            nc.scalar.dma_start(t, wo[pair * 128:(pair + 1) * 128, :])
            wo_sb.append(t)
        # stage B (xt cols 512:1024, needed by qt piece 1) then stage C
        for lo, hi in ((512, 1024), (1024, _N)):
            for kc in range(_KC):
                (nc.sync if kc < 2 else nc.gpsimd).dma_start(
                    xt_sb[kc][:, lo:hi], xt[kc * 128:(kc + 1) * 128, lo:hi])

        # Warm the PE HAM clock with dummy matmuls while input DMAs land
        # (needs ~3.4us of sustained activity to reach 2.4GHz).
        psw = psA.tile([128, 512], f32, tag="mm")
        for _ in range(10):
            nc.tensor.matmul(psw, lhsT=dummy[:, 0:128], rhs=dummy,
                             start=True, stop=True)

        # ---- qkv projections --------------------------------------------
        qt_sb = [None, None]
        kt_sb = [None, None]

        def project_qk_piece(name, wsb, lst, hc, tp, pool=None, tag="mm"):
            if lst[hc] is None:
                lst[hc] = qkp.tile([128, _N], bf16, tag=f"{name}{hc}",
                                   name=f"{name}{hc}")
            dst = lst[hc]
            ps = (pool or psA).tile([128, 512], f32, tag=tag)
            for kc in range(_KC):
                nc.tensor.matmul(
                    ps,
                    lhsT=wsb[kc][:, hc * 128:(hc + 1) * 128],
                    rhs=xt_sb[kc][:, tp * 512:(tp + 1) * 512],
                    start=(kc == 0), stop=(kc == _KC - 1),
                )
            nc.vector.tensor_copy(dst[:, tp * 512:(tp + 1) * 512], ps)

        # V augmented with a ones column: va[t][:, h, 0:64] = V, [..., 64]=1
        va_sb = []
        for t in range(_TC):
            va_sb.append(vap.tile([128, _HPC, 65], bf16, tag=f"va{t}",
                                  name=f"va{t}"))

        def v_piece(t, pool=None, tag="mm"):
            va = va_sb[t]
            nc.gpsimd.memset(va[:, :, 64:65], 1.0)
            ps = (pool or psA).tile([128, _HL], f32, tag=tag)
            for kc in range(_KC):
                nc.tensor.matmul(
                    ps,
                    lhsT=xt_sb[kc][:, t * 128:(t + 1) * 128],
                    rhs=wv_sb[kc],
                    start=(kc == 0), stop=(kc == _KC - 1),
                )
            nc.vector.tensor_copy(
                va[:, :, 0:64], ps.rearrange("p (h d) -> p h d", h=_HPC))

        # Minimum serial prefix gating the first exp: qt pieces 0-1 and
        # kt piece 0.  va 0-2 are emitted right after the first scores
        # (PV lags two blocks and pt has 5 buffers, so they can be late).
        # Spread across psB AND psA so no piece waits on a prior piece's
        # psum->sbuf copy (4 slots instead of 2).
        project_qk_piece("qt", wq_sb, qt_sb, 0, 0, pool=psB, tag="mo")
        project_qk_piece("kt", wk_sb, kt_sb, 0, 0, pool=psA, tag="mm")
        project_qk_piece("qt", wq_sb, qt_sb, 0, 1, pool=psB, tag="mo")

        def mkv(t):
            return lambda: v_piece(t, pool=psB, tag="mo")

        def mkp(name, wsb, lst, hc, tp):
            return lambda: project_qk_piece(name, wsb, lst, hc, tp,
                                            pool=psB, tag="mo")

        # ---- attention state --------------------------------------------
        acc = []
        for t in range(_TC):
            acc.append(outp.tile([128, _DIM], f32, tag=f"acc{t}",
                                 name=f"acc{t}"))
        at_sb = [None, None]        # per pair, [128, N] (head rows stacked)
        zrec = [None] * _HPC

        def outproj_chunk(pair, t, store, wide=False):
            h0, h1 = 2 * pair, 2 * pair + 1
            tsl = slice(t * 128, (t + 1) * 128)
            ps0 = psB.tile([128, _DIM], f32, tag="mo")
            ps1 = (psOp if wide else psB).tile(
                [128, _DIM], f32, tag="po" if wide else "mo")
            nc.tensor.matmul(ps0, lhsT=at_sb[pair][0:64, tsl],
                             rhs=wo_sb[pair][0:64, :], start=True, stop=True)
            nc.tensor.matmul(ps1, lhsT=at_sb[pair][64:128, tsl],
                             rhs=wo_sb[pair][64:128, :], start=True, stop=True)
            if h0 == 0:
                nc.vector.tensor_scalar_mul(acc[t], ps0, zrec[h0][:, t:t + 1])
            else:
                nc.vector.scalar_tensor_tensor(
                    out=acc[t], in0=ps0, scalar=zrec[h0][:, t:t + 1],
                    in1=acc[t], op0=Alu.mult, op1=Alu.add,
                )
            nc.vector.scalar_tensor_tensor(
                out=acc[t], in0=ps1, scalar=zrec[h1][:, t:t + 1],
                in1=acc[t], op0=Alu.mult, op1=Alu.add,
            )
            if store:
                (nc.gpsimd if t % 2 else nc.sync).dma_start(
                    out[tsl, :], acc[t])

        def mkop(pair, t, store, wide=False):
            return lambda: outproj_chunk(pair, t, store, wide)

        def mkz(h, zr_row, q, qs, eng):
            def zchain():
                zd = zdp.tile([1, 512], f32, tag=f"zd{h % 2}")
                eng.dma_start(zd, zr_row[0:1, qs])
                zcol = zp.tile([128, 4], f32, tag=f"zcol{h % 2}")
                eng.dma_start(
                    zcol, zd.rearrange("o (j p) -> (o p) j", p=128))
                nc.vector.reciprocal(zrec[h][:, q * 4:(q + 1) * 4], zcol)
            return zchain

        def mkz_pe(h, zr_row, q):
            def zchain():
                pz = psB.tile([128, 4], f32, tag="mo")
                for j in range(4):
                    jj = q * 4 + j
                    nc.tensor.transpose(
                        pz[:, j:j + 1],
                        zr_row[0:1, jj * 128:(jj + 1) * 128], ones11)
                nc.vector.reciprocal(zrec[h][:, q * 4:(q + 1) * 4], pz)
            return zchain

        # ---- per-quarter pending-op schedules ---------------------------
        # quarter key (pair, q); each entry is a list of thunks popped one
        # per ec-block inside that quarter.
        nop = lambda: None  # noqa: E731
        sched = {}
        # pair0 q0: V ramp + remaining qt/kt pieces for pair 0.
        sched[(0, 0)] = [mkv(3), mkv(4), mkp("qt", wq_sb, qt_sb, 0, 2),
                         mkv(5), mkv(6), mkp("qt", wq_sb, qt_sb, 0, 3),
                         mkv(7), mkv(8), mkv(9),
                         mkp("kt", wk_sb, kt_sb, 0, 1),
                         mkv(10), mkv(11), mkv(12), mkv(13), mkv(14),
                         mkv(15)]
        # later quarters: the drain prepends z-chains + outproj of the
        # previous quarter, interleaved with these static pieces so no
        # two heavy PE pops land in adjacent blocks.
        sched[(0, 1)] = [mkp("kt", wk_sb, kt_sb, 0, 2),
                         mkp("kt", wk_sb, kt_sb, 1, 0),
                         mkp("qt", wq_sb, qt_sb, 1, 0),
                         mkp("qt", wq_sb, qt_sb, 1, 1)]
        sched[(0, 2)] = [mkp("kt", wk_sb, kt_sb, 0, 3),
                         mkp("qt", wq_sb, qt_sb, 1, 2),
                         mkp("qt", wq_sb, qt_sb, 1, 3),
                         mkp("kt", wk_sb, kt_sb, 1, 1)]
        sched[(0, 3)] = [mkp("kt", wk_sb, kt_sb, 1, 2)]
        sched[(1, 0)] = [mkp("kt", wk_sb, kt_sb, 1, 3)]
        sched[(1, 1)] = []
        sched[(1, 2)] = []
        sched[(1, 3)] = []

        def merge_quarter(zops, ops, statics):
            # [z0, z1, s0, op0, s1, op1, ...] - pieces spaced 2 apart,
            # first op ~3 blocks after its z-chain is issued.
            lst = list(zops)
            a, b = list(statics), list(ops)
            while a or b:
                if a:
                    lst.append(a.pop(0))
                if b:
                    lst.append(b.pop(0))
            return lst

        # ---- main loop: software-pipelined blocks ------------------------
        blocks = [(p, q, ec) for p in range(2) for q in range(4)
                  for ec in range(_TC)]

        for p in range(2):
            at_sb[p] = atp.tile([128, _N], bf16, tag=f"at{p}", name=f"at{p}")
            for h in (2 * p, 2 * p + 1):
                zrec[h] = zp.tile([128, _TC], f32, tag=f"zrec{h}",
                                  name=f"zrec{h}", bufs=1)
        zrow = {}
        for p in range(2):
            for h in range(2):
                zrow[(p, h)] = zp.tile([1, _N], f32, tag=f"zrow{p}{h}",
                                       name=f"zrow{p}{h}", bufs=1)

        def emit_scores(blk, ps):
            p, q, ec = blk
            ncol = q * 512
            nc.tensor.matmul(
                ps[:, 0:512],
                lhsT=qt_sb[p][0:64, ec * 128:(ec + 1) * 128],
                rhs=kt_sb[p][0:64, ncol:ncol + 512],
                start=True, stop=True,
            )
            nc.tensor.matmul(
                ps[:, 512:1024],
                lhsT=qt_sb[p][64:128, ec * 128:(ec + 1) * 128],
                rhs=kt_sb[p][64:128, ncol:ncol + 512],
                start=True, stop=True,
            )

        def emit_pv(blk, pt, po0, po1):
            p, q, ec = blk
            h0, h1 = 2 * p, 2 * p + 1
            nc.tensor.matmul(
                po0[0:65, :], lhsT=va_sb[ec][:, h0, :], rhs=pt[:, 0:512],
                start=(ec == 0), stop=(ec == _TC - 1),
            )
            nc.tensor.matmul(
                po1[0:65, :], lhsT=va_sb[ec][:, h1, :], rhs=pt[:, 512:1024],
                start=(ec == 0), stop=(ec == _TC - 1),
            )

        def emit_drain(p, q, po0, po1, last_q):
            # quarter drain: out^T rows -> at_sb, Z rows -> zrow
            qs = slice(q * 512, (q + 1) * 512)
            if last_q:
                nc.vector.tensor_copy(zrow[(p, 0)][:, qs], po0[64:65, :])
                nc.scalar.activation(at_sb[p][0:64, qs], po0[0:64, :], Copy)
                nc.vector.tensor_copy(zrow[(p, 1)][:, qs], po1[64:65, :])
                nc.scalar.activation(at_sb[p][64:128, qs], po1[0:64, :], Copy)
            else:
                nc.vector.tensor_copy(at_sb[p][0:64, qs], po0[0:64, :])
                nc.vector.tensor_copy(zrow[(p, 0)][:, qs], po0[64:65, :])
                nc.vector.tensor_copy(at_sb[p][64:128, qs], po1[0:64, :])
                nc.vector.tensor_copy(zrow[(p, 1)][:, qs], po1[64:65, :])

        # scores for block 0 are part of the prefix
        ps_cur = psA.tile([128, 1024], f32, tag="mm")
        emit_scores(blocks[0], ps_cur)
        v_piece(0, pool=psB, tag="mo")
        v_piece(1, pool=psA, tag="mm")
        v_piece(2, pool=psB, tag="mo")

        # pipeline: block g emits scores(g+1), exp(g), PV(g-2), 1 pop.
        # PV lagging TWO blocks means every PV semaphore is pre-satisfied
        # when the in-order PE reaches it, so the PV chain streams
        # seamlessly after the scores chain (no 173ns pipeline refill).
        from collections import deque
        po_cur = None
        inflight = deque()
        pt_early = None
        pending = list(sched[(0, 0)])

        def retire(entry):
            blkp, ptp_, pop_ = entry
            emit_pv(blkp, ptp_, *pop_)
            if blkp[2] == _TC - 1:
                emit_drain(blkp[0], blkp[1], *pop_, last_q=False)
                pq, qq = blkp[0], blkp[1]
                h0, h1 = 2 * pq, 2 * pq + 1
                qs = slice(qq * 512, (qq + 1) * 512)
                zops = [mkz(h0, zrow[(pq, 0)], qq, qs, nc.sync),
                        mkz(h1, zrow[(pq, 1)], qq, qs, nc.gpsimd)]
                ops = [mkop(pq, t, store=(pq == 1))
                       for t in range(qq * 4, (qq + 1) * 4)]
                nq = (pq, qq + 1) if qq < 3 else (pq + 1, 0)
                return merge_quarter(zops, ops, sched[nq])
            return None

        for g, blk in enumerate(blocks):
            p, q, ec = blk
            if ec == 0:
                po_cur = (psOp.tile([65, 512], f32, tag="po", name="po0"),
                          psOp.tile([65, 512], f32, tag="po", name="po1"))

            # 1) scores for the NEXT block (PE clears ACT's next dep early)
            if g + 1 < len(blocks):
                ps_next = psA.tile([128, 1024], f32, tag="mm")
                emit_scores(blocks[g + 1], ps_next)
            # 2) exp of the current block.  DVE integer-exp blocks were
            #    already emitted one block early (queue-latency headroom).
            if pt_early is not None:
                pt = pt_early
                pt_early = None
            else:
                pt = ptp.tile([128, 1024], bf16, tag="pt")
                nc.scalar.activation(pt, ps_cur, Exp, scale=_SCALE)
            if g + 1 < len(blocks) and blocks[g + 1][2] in _DVE_ECS:
                pt_early = ptp.tile([128, 1024], bf16, tag="pt",
                                    name="pt_early")
                nc.vector.tensor_scalar(
                    out=pt_early.bitcast(i16), in0=ps_next,
                    scalar1=float(_EXPA), scalar2=float(_EXPB),
                    op0=Alu.mult, op1=Alu.add,
                )
            ps_cur = ps_next
            # 3) PV for the block TWO back
            inflight.append((blk, pt, po_cur))
            if len(inflight) > 2:
                upd = retire(inflight.popleft())
                if upd is not None:
                    pending = upd + pending
            # 4) one deferred op
            if pending:
                pending.pop(0)()

        # final blocks' PVs, then a 128-col-sliced drain/outproj pipeline:
        # slice j's drain copies (ACT), z transpose+recip, outproj matmuls,
        # RMWs (DVE) and store all overlap slice j+1's.
        e1 = inflight.popleft()
        emit_pv(e1[0], e1[1], *e1[2])
        e2 = inflight.popleft()
        emit_pv(e2[0], e2[1], *e2[2])
        po0, po1 = e2[2]
        for j in range(4):
            t = 12 + j
            jsl = slice(j * 128, (j + 1) * 128)
            qsl = slice(1536 + j * 128, 1536 + (j + 1) * 128)
            nc.vector.tensor_copy(zrow[(1, 0)][:, qsl], po0[64:65, jsl])
            nc.vector.tensor_copy(zrow[(1, 1)][:, qsl], po1[64:65, jsl])
            nc.scalar.activation(at_sb[1][0:64, qsl], po0[0:64, jsl], Copy)
            nc.scalar.activation(at_sb[1][64:128, qsl], po1[0:64, jsl], Copy)
            pz = psA.tile([128, 2], f32, tag="mm", name="pz")
            nc.tensor.transpose(pz[:, 0:1], zrow[(1, 0)][0:1, qsl], ones11)
            nc.tensor.transpose(pz[:, 1:2], zrow[(1, 1)][0:1, qsl], ones11)
            nc.vector.reciprocal(zrec[2][:, t:t + 1], pz[:, 0:1])
            nc.vector.reciprocal(zrec[3][:, t:t + 1], pz[:, 1:2])
            outproj_chunk(1, t, store=True)
        while pending:
            pending.pop(0)()


def _build():
    if "/opt/trn_rl_repo" not in sys.path:
        sys.path.insert(0, "/opt/trn_rl_repo")
    from concourse import bacc, mybir
    import concourse.tile as tile

    dt = mybir.dt
    nc = bacc.Bacc("TRN2", target_bir_lowering=False, debug=False,
                   num_devices=_NCORES)
    xt = nc.dram_tensor("xt", [_DIM, _N], dt.bfloat16, kind="ExternalInput").ap()
    wq = nc.dram_tensor("wq", [_DIM, _HL], dt.bfloat16, kind="ExternalInput").ap()
    wk = nc.dram_tensor("wk", [_DIM, _HL], dt.bfloat16, kind="ExternalInput").ap()
    wv = nc.dram_tensor("wv", [_DIM, _HL], dt.bfloat16, kind="ExternalInput").ap()
    wo = nc.dram_tensor("wo", [_HL, _DIM], dt.bfloat16, kind="ExternalInput").ap()
    out = nc.dram_tensor("out", [_N, _DIM], dt.float32, kind="ExternalOutput").ap()

    with tile.TileContext(nc) as tc:
        _emit(tc, xt, wq, wk, wv, wo, out, mybir)
    nc.compile()
    return nc


def _get_nc():
    if "nc" not in _cache:
        _cache["nc"] = _build()
    return _cache["nc"]


def _shard_inputs(X, W_qkv, W_out):
    import ml_dtypes
    bf16 = ml_dtypes.bfloat16
    in_maps = []
    for c in range(_NCORES):
        b, g = c // 2, c % 2
        cols = slice(g * _HL, (g + 1) * _HL)
        in_maps.append({
            "xt": np.ascontiguousarray(X[b].T).astype(bf16),
            "wq": W_qkv[:, 0 * _DIM:][:, cols].astype(bf16),
            "wk": W_qkv[:, 1 * _DIM:][:, cols].astype(bf16),
            "wv": W_qkv[:, 2 * _DIM:][:, cols].astype(bf16),
            "wo": W_out[g * _HL:(g + 1) * _HL, :].astype(bf16),
        })
    return in_maps


def _run(inputs, trace=False):
    if "/opt/trn_rl_repo" not in sys.path:
        sys.path.insert(0, "/opt/trn_rl_repo")
    from concourse.bass_utils import run_bass_kernel_spmd

    X = np.asarray(inputs["X"], dtype=np.float32)
    W_qkv = np.asarray(inputs["W_qkv"], dtype=np.float32)
    W_out = np.asarray(inputs["W_out"], dtype=np.float32)
    b_out = np.asarray(inputs["b_out"], dtype=np.float32)

    nc = _get_nc()
    in_maps = _shard_inputs(X, W_qkv, W_out)
    res = run_bass_kernel_spmd(nc, in_maps, list(range(_NCORES)), trace=trace)

    out = np.empty((_B, _N, _DIM), dtype=np.float32)
    for b in range(_B):
        out[b] = res.results[2 * b]["out"] + res.results[2 * b + 1]["out"] + b_out
    return out, res.exec_time_ns


def kernel(**inputs) -> np.ndarray:
    out, _ = _run(inputs, trace=False)
    return out
